# revision 12
# baseline (speedup 1.0000x reference)
"""Trainium2 Bass kernel for nn_Attention (B=1, S=2048, D=4096, H=32, KVH=8).

Sharding: tensor-parallel over heads across 8 cores (4 Q heads + 1 KV head
per core). Each core computes QKV projections for its heads from the full
x^T, applies RoPE, runs causal flash attention in transposed layout, then an
AllToAll redistributes attention outputs so each core computes the final
out-projection rows for its 1/8 of the sequence against the full wo. The
host concatenates the per-core row blocks.

Matmuls run as float32r (full PE rate for N>=256, fp32 storage).
"""

import sys

for _p in ("/opt/trn_rl_repo",):
    if _p not in sys.path:
        sys.path.insert(0, _p)

from contextlib import ExitStack
from math import sqrt

import numpy as np

import concourse.bass as bass
import concourse.tile as tile
from concourse import bacc, mybir
from concourse.masks import make_identity

F32 = mybir.dt.float32
F32R = mybir.dt.float32r

FULL_CFG = dict(S=2048, D=4096, H=32, KVH=8, HD=128, NB=512, n_cores=8)


def build_bass(cfg):
    """Build the SPMD per-core Bass program. Same program on every core; all
    per-core differences live in the input data."""
    S, D, H, KVH, HD = cfg["S"], cfg["D"], cfg["H"], cfg["KVH"], cfg["HD"]
    NB, NC = cfg["NB"], cfg["n_cores"]
    HPC = H // NC          # q heads per core
    KC = D // 128          # contraction chunks for projections
    NSB = S // NB          # seq blocks
    NKJ = NB // 128        # kj 128-blocks per seq block
    R = S // NC            # output rows per core
    MT = R // 128          # output row tiles
    NO = D // 512          # out-proj column blocks
    GH = H // 1            # total global heads
    scale = 1.0 / sqrt(HD)

    assert NB % R == 0 or R % NB == 0
    PIECES = max(1, NB // R)   # a2a pieces per (head, seq block)

    nc = bacc.Bacc(
        "TRN2",
        target_bir_lowering=False,
        debug=False,
        enable_asserts=False,
        num_devices=NC,
    )

    xT = nc.dram_tensor("xT", [D, S], F32R, kind="ExternalInput").ap()
    wq = nc.dram_tensor("wq", [D, HPC * HD], F32R, kind="ExternalInput").ap()
    wk = nc.dram_tensor("wk", [D, HD], F32R, kind="ExternalInput").ap()
    wv = nc.dram_tensor("wv", [D, HD], F32R, kind="ExternalInput").ap()
    wo = nc.dram_tensor("wo", [H * HD, D], F32R, kind="ExternalInput").ap()
    cosT = nc.dram_tensor("cosT", [HD // 2, S], F32, kind="ExternalInput").ap()
    sinT = nc.dram_tensor("sinT", [HD // 2, S], F32, kind="ExternalInput").ap()
    out = nc.dram_tensor("out", [R, D], F32, kind="ExternalOutput").ap()

    with tile.TileContext(nc) as tc, ExitStack() as ctx:
        # ---- pools that span phases ----
        persist = ctx.enter_context(tc.tile_pool(name="persist", bufs=1))
        dram = ctx.enter_context(tc.tile_pool(name="dram", bufs=1, space="DRAM"))

        QT = [persist.tile([128, S], F32R, tag=f"qt{h}", name=f"QT{h}")
              for h in range(HPC)]
        KT = persist.tile([128, S], F32R, tag="kt", name="KT")
        Vn = persist.tile([128, S], F32R, tag="vn", name="Vn")  # V in [seq, hd] 128-tiles
        cos_sb = persist.tile([HD // 2, S], F32, tag="cos", name="cos_sb")
        sin_sb = persist.tile([HD // 2, S], F32, tag="sin", name="sin_sb")
        ident = persist.tile([128, 128], F32, tag="ident", name="ident")
        ones_f = persist.tile([128, 128], F32, tag="ones_f", name="ones_f")
        ones = persist.tile([128, 128], F32R, tag="ones", name="ones")

        nc.sync.dma_start(out=cos_sb, in_=cosT)
        nc.sync.dma_start(out=sin_sb, in_=sinT)
        make_identity(nc, ident)
        nc.vector.memset(ones_f, 1.0)
        nc.vector.tensor_copy(ones, ones_f)

        a2a_in = [dram.tile([NC, 128, R], F32R, tag=f"ain{h}", name=f"ain{h}")
                  for h in range(HPC)]
        a2a_out = [dram.tile([NC, 128, R], F32R, tag=f"aout{h}", name=f"aout{h}")
                   for h in range(HPC)]

        # ================= Phase 1: QKV projection + RoPE =================
        with (
            tc.tile_pool(name="wgt", bufs=1) as wgt,
            tc.tile_pool(name="xtp", bufs=4) as xtp,
            tc.tile_pool(name="vt_sb", bufs=1) as vtp,
            tc.tile_pool(name="pj_ps", bufs=1, space="PSUM") as pjps,
            tc.tile_pool(name="rope_t", bufs=4) as ropep,
        ):
            wq_sb = wgt.tile([128, KC, HPC * HD], F32R, tag="wq", name="wq_sb")
            wk_sb = wgt.tile([128, KC, HD], F32R, tag="wk", name="wk_sb")
            wv_sb = wgt.tile([128, KC, HD], F32R, tag="wv", name="wv_sb")
            nc.sync.dma_start(out=wq_sb, in_=wq.rearrange("(k p) m -> p k m", p=128))
            nc.sync.dma_start(out=wk_sb, in_=wk.rearrange("(k p) m -> p k m", p=128))
            nc.sync.dma_start(out=wv_sb, in_=wv.rearrange("(k p) m -> p k m", p=128))

            VT_sb = vtp.tile([128, S], F32, tag="vt", name="VT_sb")

            # outputs: (lhsT chunk selector, is_rope, dest row of SBUF)
            def proj_outputs():
                outs = []
                for h in range(HPC):
                    outs.append((lambda k, h=h: wq_sb[:, k, h * HD:(h + 1) * HD],
                                 "q", h))
                outs.append((lambda k: wk_sb[:, k, :], "k", 0))
                outs.append((lambda k: wv_sb[:, k, :], "v", 0))
                return outs

            outs = proj_outputs()
            groups = [outs[: (len(outs) + 1) // 2], outs[(len(outs) + 1) // 2:]]

            for n in range(NSB):
                nsl = slice(n * NB, (n + 1) * NB)
                for grp in groups:
                    x_chunks = []
                    for k in range(KC):
                        xc = xtp.tile([128, NB], F32R, tag="x", name="xc")
                        nc.sync.dma_start(out=xc, in_=xT[k * 128:(k + 1) * 128, nsl])
                        x_chunks.append(xc)
                    for wsel, kind, h in grp:
                        ps = pjps.tile([128, NB], F32, tag=f"p{kind}{h}", name="ps")
                        for k in range(KC):
                            nc.tensor.matmul(
                                ps,
                                lhsT=wsel(k),
                                rhs=x_chunks[k],
                                start=(k == 0),
                                stop=(k == KC - 1),
                            )
                        if kind == "v":
                            nc.scalar.copy(VT_sb[:, nsl], ps)
                        else:
                            dst = KT if kind == "k" else QT[h]
                            hw = HD // 2
                            E, O = ps[0:hw, :], ps[hw:2 * hw, :]
                            c, s = cos_sb[:, nsl], sin_sb[:, nsl]
                            t1 = ropep.tile([hw, NB], F32, tag="rt", name="t1")
                            t2 = ropep.tile([hw, NB], F32, tag="rt", name="t2")
                            nc.vector.tensor_mul(t1, E, c)
                            nc.vector.tensor_mul(t2, O, s)
                            nc.vector.tensor_sub(dst[0:hw, nsl], t1, t2)
                            t3 = ropep.tile([hw, NB], F32, tag="rt", name="t3")
                            t4 = ropep.tile([hw, NB], F32, tag="rt", name="t4")
                            nc.vector.tensor_mul(t3, E, s)
                            nc.vector.tensor_mul(t4, O, c)
                            nc.vector.tensor_add(dst[hw:2 * hw, nsl], t3, t4)

            # transpose VT [hd, seq] -> Vn tiles [seq, hd]
            with tc.tile_pool(name="tps", bufs=2, space="PSUM") as tps:
                for t in range(S // 128):
                    tsl = slice(t * 128, (t + 1) * 128)
                    pst = tps.tile([128, 128], F32, tag="tp", name="pst")
                    nc.tensor.transpose(pst, VT_sb[:, tsl], ident)
                    nc.scalar.copy(Vn[:, tsl], pst)

        # ================= Phase 2: causal flash attention ================
        with (
            tc.tile_pool(name="sc_ps", bufs=3, space="PSUM") as scps,
            tc.tile_pool(name="o_ps", bufs=2, space="PSUM") as ops_,
            tc.tile_pool(name="rs_ps", bufs=2, space="PSUM") as rsps,
            tc.tile_pool(name="exp_sb", bufs=4) as exps,
            tc.tile_pool(name="att_sb", bufs=4) as atts,
        ):
            for h in range(HPC):
                for n in range(NSB):
                    nsl = slice(n * NB, (n + 1) * NB)
                    o_ps = ops_.tile([128, NB], F32, tag="o", name="o_ps")
                    rs_ps = rsps.tile([128, NB], F32, tag="rs", name="rs_ps")
                    q_rhs = QT[h][:, nsl]
                    nkj = (n + 1) * NKJ
                    pending = None  # software-pipeline the PV/rowsum matmuls
                    for j in range(nkj):
                        jsl = slice(j * 128, (j + 1) * 128)
                        sc = scps.tile([128, NB], F32, tag="sc", name="sc")
                        nc.tensor.matmul(
                            sc, lhsT=KT[:, jsl], rhs=q_rhs,
                            start=True, stop=True,
                        )
                        ex = exps.tile([128, NB], F32R, tag="ex", name="ex")
                        nc.scalar.activation(
                            ex, sc, mybir.ActivationFunctionType.Exp, scale=scale
                        )
                        d = j - n * NKJ
                        if d >= 0:  # block overlaps the causal diagonal
                            nc.gpsimd.affine_select(
                                out=ex, in_=ex,
                                compare_op=mybir.AluOpType.is_ge,
                                fill=0.0, base=-d * 128,
                                pattern=[[1, NB]], channel_multiplier=-1,
                            )
                        if pending is not None:
                            pj, pex = pending
                            psl = slice(pj * 128, (pj + 1) * 128)
                            nc.tensor.matmul(
                                o_ps, lhsT=Vn[:, psl],
                                rhs=pex,
                                start=(pj == 0), stop=False,
                            )
                            nc.tensor.matmul(
                                rs_ps, lhsT=ones,
                                rhs=pex,
                                start=(pj == 0), stop=False,
                            )
                        pending = (j, ex)
                    pj, pex = pending
                    psl = slice(pj * 128, (pj + 1) * 128)
                    nc.tensor.matmul(
                        o_ps, lhsT=Vn[:, psl], rhs=pex,
                        start=(pj == 0), stop=True,
                    )
                    nc.tensor.matmul(
                        rs_ps, lhsT=ones, rhs=pex,
                        start=(pj == 0), stop=True,
                    )
                    rcp = atts.tile([128, NB], F32, tag="rcp", name="rcp")
                    nc.vector.reciprocal(rcp, rs_ps)
                    o_sb = atts.tile([128, NB], F32R, tag="osb", name="o_sb")
                    nc.vector.tensor_mul(o_sb, o_ps, rcp)
                    for jj in range(PIECES):
                        piece_idx = n * PIECES + jj
                        nc.sync.dma_start(
                            out=a2a_in[h][piece_idx],
                            in_=o_sb[:, jj * R:(jj + 1) * R],
                        )
                nc.gpsimd.collective_compute(
                    "AllToAll",
                    mybir.AluOpType.bypass,
                    replica_groups=[list(range(NC))],
                    ins=[a2a_in[h].opt()],
                    outs=[a2a_out[h].opt()],
                )

        # ================= Phase 3: out-projection ========================
        with (
            tc.tile_pool(name="pc_sb", bufs=1) as pcp,
            tc.tile_pool(name="wo_sb", bufs=8) as wop,
            tc.tile_pool(name="op_ps", bufs=2, space="PSUM") as opps,
            tc.tile_pool(name="ob_sb", bufs=4) as obp,
        ):
            piece = {}
            for h in range(HPC):
                for i in range(NC):
                    t = pcp.tile([128, R], F32R, tag=f"pc{h}_{i}", name=f"pc{h}_{i}")
                    nc.sync.dma_start(out=t, in_=a2a_out[h][i])
                    piece[(h, i)] = t

            for no in range(NO):
                osl = slice(no * 512, (no + 1) * 512)
                pso = [opps.tile([128, 512], F32, tag=f"po{m}", name=f"pso{m}")
                       for m in range(MT)]
                for g in range(H):
                    i, h = divmod(g, HPC)
                    wo_t = wop.tile([128, 512], F32R, tag="wo", name="wo_t")
                    nc.sync.dma_start(out=wo_t, in_=wo[g * 128:(g + 1) * 128, osl])
                    for m in range(MT):
                        nc.tensor.matmul(
                            pso[m],
                            lhsT=piece[(h, i)][:, m * 128:(m + 1) * 128],
                            rhs=wo_t,
                            start=(g == 0),
                            stop=(g == H - 1),
                        )
                for m in range(MT):
                    ob = obp.tile([128, 512], F32, tag="ob", name="ob")
                    nc.scalar.copy(ob, pso[m])
                    nc.sync.dma_start(
                        out=out[m * 128:(m + 1) * 128, osl], in_=ob
                    )

    nc.compile()
    return nc


def prep_inputs(cfg, x, wq, wk, wv, wo, freqs_cos, freqs_sin):
    """Host-side sharding/layout prep. Returns list of per-core input dicts."""
    S, D, H, KVH, HD, NC = (cfg["S"], cfg["D"], cfg["H"], cfg["KVH"], cfg["HD"],
                            cfg["n_cores"])
    HPC = H // NC
    x = np.asarray(x, np.float32).reshape(S, D)
    wq = np.asarray(wq, np.float32)
    wk = np.asarray(wk, np.float32)
    wv = np.asarray(wv, np.float32)
    wo = np.asarray(wo, np.float32)
    cos = np.asarray(freqs_cos, np.float32)
    sin = np.asarray(freqs_sin, np.float32)

    xT = np.ascontiguousarray(x.T)                      # [D, S]
    cosT = np.ascontiguousarray(cos.T)                  # [HD/2, S]
    sinT = np.ascontiguousarray(sin.T)

    # de-interleave rope pairs: new col i <- 2i, new col i+HD/2 <- 2i+1
    idx = np.concatenate([np.arange(0, HD, 2), np.arange(1, HD, 2)])
    wq_p = wq.reshape(D, H, HD)[:, :, idx]
    wk_p = wk.reshape(D, KVH, HD)[:, :, idx]
    wv_r = wv.reshape(D, KVH, HD)

    in_maps = []
    for c in range(NC):
        kv = c * KVH // NC
        in_maps.append(dict(
            xT=xT,
            wq=np.ascontiguousarray(
                wq_p[:, c * HPC:(c + 1) * HPC].reshape(D, HPC * HD)),
            wk=np.ascontiguousarray(wk_p[:, kv]),
            wv=np.ascontiguousarray(wv_r[:, kv]),
            wo=wo,
            cosT=cosT,
            sinT=sinT,
        ))
    return in_maps


_CACHED = {}


def _get_nc(cfg_key=None):
    if "nc" not in _CACHED:
        _CACHED["nc"] = build_bass(FULL_CFG)
    return _CACHED["nc"]


def run_spmd(x, wq, wk, wv, wo, freqs_cos, freqs_sin, **spmd_kwargs):
    """Build (cached), run on 8 cores, return (full_output, BassKernelResults)."""
    from concourse.bass_utils import run_bass_kernel_spmd

    cfg = FULL_CFG
    NC = cfg["n_cores"]
    in_maps = prep_inputs(cfg, x, wq, wk, wv, wo, freqs_cos, freqs_sin)
    nc = _get_nc()
    res = run_bass_kernel_spmd(nc, in_maps, list(range(NC)), **spmd_kwargs)
    parts = [res.results[c]["out"] for c in range(NC)]
    full = np.concatenate(parts, axis=0)
    return full.reshape(1, cfg["S"], cfg["D"]).astype(np.float32), res


def kernel(x, wq, wk, wv, wo, freqs_cos, freqs_sin):
    out, _ = run_spmd(x, wq, wk, wv, wo, freqs_cos, freqs_sin)
    return out
